# revision 36
# baseline (speedup 1.0000x reference)
"""DynamicMaskHead Trainium2 kernel (b5).

Per-instance 3-layer MLP over pixels (grouped 1x1 convs):
    out = w2 @ relu(w1 @ relu(w0 @ x + b0) + b1) + b2
with 128 instances, x: [10, 25600] per instance.
16 instances per NeuronCore (8 cores, data-parallel).

Structure (per core, instances j in [0,16)):
  - L1: K=128 block-diagonal matmul (cin 0..7) + accumulating K=32 strip
    matmul (cin 8..9) on PE row strip 32*(t%4) via tile_position; strips
    of a 3-tile group are distinct so the small matmuls overlap on the
    PE's independent sub-arrays.
  - L2: K=128 block-diagonal matmul; bias+relu epilogue on VectorE.
  - L3: K=128 -> M=32 matmul on PE column strip 32*(t%4) using
    zero-padded w3 halves (h=(t//4)%2) so 8 consecutive tiles accumulate
    into ONE full 128-partition PSUM bank; one IDENTITY (bias) epilogue
    per 8 tiles writes bf16 into a resident output tile.
  - Output: accumulated entirely in SBUF (scrambled strip layout),
    drained at the end with ONE fully-contiguous DMA; the host
    unscrambles the permutation and casts back to f32 (both free).
  - x2 (cin 8..9) is pre-packed on the host into its strip layout so it
    loads with contiguous DMAs; x1/x2 stream lazily per super-tile in
    need order (DMA waits share coalesced semaphore lanes, so issue
    order must track need order).
  - Matmul operands bf16 (fp32 PSUM accumulate), rounded on the host so
    the HBM stream is half-width.
"""

import sys

if "/opt/trn_rl_repo" not in sys.path:
    sys.path.insert(0, "/opt/trn_rl_repo")

import ml_dtypes
import numpy as np

N_CORES = 8
N_INST = 128
C_IN = 10
C = 8
H = W = 160
P = H * W          # 25600 pixels
PER = N_INST // N_CORES  # 16 instances per core
F = 512            # pixels per matmul tile (one fp32 PSUM bank)
NTILE = P // F     # 50
SUP = 5120         # pixels per x super-tile
NSUP = P // SUP    # 5
TPS = SUP // F     # 10 tiles per super
NSLOT = (NTILE + 3) // 4   # 13 x2 slots per strip
NBLK = (NTILE + 7) // 8    # 7 output blocks (8 tiles each)
X2B = [0, 3, 5, 8, 10, 13]  # x2 slot piece per super (need order)

_cached_nc = None


def _build():
    from concourse import bacc, bass, mybir, tile

    nc = bacc.Bacc("TRN2", target_bir_lowering=False, debug=False)
    f32 = mybir.dt.float32
    bf16 = mybir.dt.bfloat16
    Relu = mybir.ActivationFunctionType.Relu
    Ident = mybir.ActivationFunctionType.Identity
    op_add = mybir.AluOpType.add
    op_max = mybir.AluOpType.max

    x1_d = nc.dram_tensor("x1", [128, P], bf16, kind="ExternalInput")
    x2p_d = nc.dram_tensor("x2p", [128, NSLOT * F], bf16, kind="ExternalInput")
    w1a_d = nc.dram_tensor("w1a", [128, 128], bf16, kind="ExternalInput")
    w1b_d = nc.dram_tensor("w1b", [128, 128], bf16, kind="ExternalInput")
    w2_d = nc.dram_tensor("w2", [128, 128], bf16, kind="ExternalInput")
    w3_d = nc.dram_tensor("w3", [128, 64], bf16, kind="ExternalInput")
    b0_d = nc.dram_tensor("b0", [128, 1], f32, kind="ExternalInput")
    b1_d = nc.dram_tensor("b1", [128, 1], f32, kind="ExternalInput")
    b2r_d = nc.dram_tensor("b2r", [128, 1], f32, kind="ExternalInput")
    out_d = nc.dram_tensor("out", [128, NBLK * F], bf16, kind="ExternalOutput")

    with tile.TileContext(nc) as tc:
        with (
            tc.tile_pool(name="const", bufs=1) as cpool,
            tc.tile_pool(name="xp", bufs=4) as xpool,
            tc.tile_pool(name="hp", bufs=10) as hpool,
            tc.tile_pool(name="ps1", bufs=4, space="PSUM") as pp1,
            tc.tile_pool(name="ps2", bufs=3, space="PSUM") as pp2,
            tc.tile_pool(name="ps3", bufs=1, space="PSUM") as pp3,
        ):
            w1a = cpool.tile([128, 128], bf16)
            nc.scalar.dma_start(w1a[:], w1a_d[:])
            w1b = cpool.tile([128, 128], bf16)
            nc.scalar.dma_start(w1b[:], w1b_d[:])
            w2t = cpool.tile([128, 128], bf16)
            nc.scalar.dma_start(w2t[:], w2_d[:])
            w3t = cpool.tile([128, 64], bf16, name="w3t")
            nc.scalar.dma_start(w3t[:], w3_d[:])
            b0t = cpool.tile([128, 1], f32)
            nc.scalar.dma_start(b0t[:], b0_d[:])
            b1t = cpool.tile([128, 1], f32)
            nc.scalar.dma_start(b1t[:], b1_d[:])
            b2rt = cpool.tile([128, 1], f32)
            nc.scalar.dma_start(b2rt[:], b2r_d[:])

            # PE warm-up: dummy matmuls while the first x DMAs are in
            # flight, so HAM un-throttles before real work.
            wdum = cpool.tile([128, 128], bf16, name="wdum")
            nc.gpsimd.memset(wdum[:], 0.0)
            xdum = cpool.tile([128, F], bf16, name="xdum")
            nc.gpsimd.memset(xdum[:], 0.0)
            for wi in range(8):
                psw = pp1.tile([128, F], f32, name="psw", tag="ps1")
                nc.tensor.matmul(
                    psw[:], wdum[:], xdum[:], start=True, stop=True
                )

            # resident output accumulator (bf16, strip layout)
            obig = cpool.tile([128, NBLK * F], bf16, name="obig")

            # x super-tiles on demand: x1 halves with the super's x2 slot
            # piece in between (need order on the SP queue)
            x2t = cpool.tile([128, NSLOT * F], bf16, name="x2t")
            xtiles = {}

            def get_x(s):
                if s not in xtiles:
                    x1 = xpool.tile([128, SUP], bf16, name="x1", tag="x1")
                    # super 0 loads in quarters so the first matmul's data
                    # lands as early as possible
                    parts = 4 if s == 0 else 2
                    step = SUP // parts
                    a, b = X2B[s], X2B[s + 1]
                    for pi in range(parts):
                        lo = pi * step
                        nc.sync.dma_start(
                            x1[:, lo : lo + step],
                            x1_d[:, s * SUP + lo : s * SUP + lo + step],
                        )
                        if pi == 0:
                            # first x2 slot lands before the rest so the
                            # first group's L1b isn't starved
                            mid = a + 1 if s == 0 else b
                            nc.sync.dma_start(
                                x2t[:, a * F : mid * F],
                                x2p_d[:, a * F : mid * F],
                            )
                        elif pi == 1 and s == 0:
                            nc.sync.dma_start(
                                x2t[:, (a + 1) * F : b * F],
                                x2p_d[:, (a + 1) * F : b * F],
                            )
                    xtiles[s] = x1
                return xtiles[s]

            def xcol(t):
                s, r = divmod(t, TPS)
                return get_x(s), r

            # 3-stage software pipeline over 3-tile groups
            NG = (NTILE + 2) // 3
            st = {}

            def stage_l1(g):
                tiles = list(range(3 * g, min(3 * g + 3, NTILE)))
                get_x(tiles[-1] // TPS)
                ps1s = {}
                for t in tiles:
                    ps1s[t] = pp1.tile([128, F], f32, name="ps1", tag="ps1")
                for t in tiles:
                    x1, r = xcol(t)
                    nc.tensor.matmul(
                        ps1s[t][:], w1a[:], x1[:, bass.ts(r, F)],
                        start=True, stop=False,
                    )
                for t in tiles:
                    s, q = t % 4, t // 4
                    nc.tensor.matmul(
                        ps1s[t][:],
                        w1b[32 * s : 32 * s + 32, :],
                        x2t[32 * s : 32 * s + 32, bass.ts(q, F)],
                        start=False, stop=True,
                        tile_position=(32 * s, 0),
                    )
                h1s = {}
                for t in tiles:
                    h1 = hpool.tile([128, F], bf16, name="h1", tag="h1", bufs=5)
                    nc.scalar.activation(h1[:], ps1s[t][:], Relu, bias=b0t[:])
                    h1s[t] = h1
                st[g] = {"h1s": h1s}

            def stage_l2(g):
                h1s = st[g]["h1s"]
                h2s = {}
                for t in sorted(h1s):
                    ps2 = pp2.tile([128, F], f32, name="ps2", tag="ps2")
                    nc.tensor.matmul(
                        ps2[:], w2t[:], h1s[t][:], start=True, stop=True
                    )
                    h2s[t] = (hpool.tile([128, F], bf16, name="h2", tag="h2"), ps2)
                for t in sorted(h2s):
                    h2, ps2 = h2s[t]
                    nc.vector.tensor_scalar(
                        h2[:], ps2[:], b1t[:], 0.0, op0=op_add, op1=op_max
                    )
                st[g]["h2s"] = {t: v[0] for t, v in h2s.items()}

            # ps3 blocks: 8 consecutive tiles accumulate into one bank;
            # tile t -> rows 32*(t%4) + 16*((t//4)%2) via zero-padded w3
            # halves. One IDENTITY epilogue per block into obig.
            ps3blk = {}

            def stage_l3(g):
                h2s = st[g]["h2s"]
                for t in sorted(h2s):
                    c, s, h = t // 8, t % 4, (t // 4) % 2
                    if c not in ps3blk:
                        ps3blk[c] = pp3.tile([128, F], f32, name="ps3", tag="ps3")
                    ps3 = ps3blk[c]
                    partner = 8 * c + 4 * (1 - h) + s
                    nc.tensor.matmul(
                        ps3[32 * s : 32 * s + 32, :],
                        w3t[:, 32 * h : 32 * h + 32],
                        h2s[t][:],
                        start=(h == 0), stop=(h == 1 or partner >= NTILE),
                        tile_position=(0, 32 * s),
                    )
                    if t == 8 * c + 7 or t == NTILE - 1:
                        blk = [tt for tt in range(8 * c, 8 * c + 8) if tt < NTILE]
                        nrows = max(
                            32 * (tt % 4) + 16 * ((tt // 4) % 2) + 16
                            for tt in blk
                        )
                        nc.scalar.activation(
                            obig[0:nrows, bass.ts(c, F)], ps3[0:nrows, :],
                            Ident, bias=b2rt[0:nrows, :],
                        )
                        del ps3blk[c]
                        # drain this block now on the idle scalar HWDGE
                        # queue (waits only on the IDENTITY above; no
                        # input DMAs behind it to stall)
                        nc.scalar.dma_start(
                            out_d[:, bass.ts(c, F)], obig[:, bass.ts(c, F)]
                        )

            for i in range(NG + 2):
                if i < NG:
                    stage_l1(i)
                if 0 <= i - 1 < NG:
                    stage_l2(i - 1)
                if 0 <= i - 2 < NG:
                    stage_l3(i - 2)
                    del st[i - 2]



    nc.compile()
    return nc


def _prep_inputs(features, params):
    feats = np.ascontiguousarray(features, dtype=np.float32).reshape(N_INST, C_IN, P)
    params = np.asarray(params, dtype=np.float32)
    bf = ml_dtypes.bfloat16
    in_maps = []
    for c in range(N_CORES):
        js = slice(c * PER, (c + 1) * PER)
        pc = params[js]
        w0 = pc[:, :80].reshape(PER, C, C_IN)
        w1 = pc[:, 80:144].reshape(PER, C, C)
        w2 = pc[:, 144:152].reshape(PER, 1, C)
        b0 = pc[:, 152:160]
        b1 = pc[:, 160:168]
        b2 = pc[:, 168:169]
        w1a = np.zeros((128, 128), np.float32)
        w1b = np.zeros((32, 128), np.float32)
        w2b = np.zeros((128, 128), np.float32)
        w3b = np.zeros((128, 64), np.float32)
        for j in range(PER):
            w1a[j * 8 : j * 8 + 8, j * 8 : j * 8 + 8] = w0[j, :, :8].T
            w1b[j * 2 : j * 2 + 2, j * 8 : j * 8 + 8] = w0[j, :, 8:10].T
            w2b[j * 8 : j * 8 + 8, j * 8 : j * 8 + 8] = w1[j].T
            # half A -> rows 32s..32s+16, half B -> rows 32s+16..32s+32
            w3b[j * 8 : j * 8 + 8, j] = w2[j, 0, :]
            w3b[j * 8 : j * 8 + 8, 48 + j] = w2[j, 0, :]
        w1br = np.tile(w1b, (4, 1))
        b2rep = np.tile(np.asarray(b2[:, 0]), 8).reshape(128, 1).astype(np.float32)
        x = feats[js]
        # x2 strip packing: tile t -> strip s=t%4 (partitions 32s+2j+cin),
        # slot q=t//4 (columns [qF,(q+1)F))
        x2src = x[:, 8:10, :].reshape(PER, 2, NTILE, F)
        x2p = np.zeros((128, NSLOT * F), np.float32)
        x2v = x2p.reshape(4, 32, NSLOT, F)
        for s in range(4):
            ts_ = list(range(s, NTILE, 4))
            x2v[s].reshape(PER, 2, NSLOT, F)[:, :, 0 : len(ts_), :] = (
                x2src[:, :, ts_, :]
            )
        in_maps.append(
            {
                "x1": np.ascontiguousarray(x[:, :8, :]).reshape(128, P).astype(bf),
                "x2p": x2p.astype(bf),
                "w1a": w1a.astype(bf),
                "w1b": w1br.astype(bf),
                "w2": w2b.astype(bf),
                "w3": w3b.astype(bf),
                "b0": np.ascontiguousarray(b0).reshape(128, 1),
                "b1": np.ascontiguousarray(b1).reshape(128, 1),
                "b2r": b2rep,
            }
        )
    return in_maps


def _run(features, params, trace=False, **kwargs):
    global _cached_nc
    from concourse.bass_utils import run_bass_kernel_spmd

    if _cached_nc is None:
        _cached_nc = _build()
    in_maps = _prep_inputs(features, params)
    res = run_bass_kernel_spmd(
        _cached_nc, in_maps, list(range(N_CORES)), trace=trace, **kwargs
    )
    out = np.empty((N_INST, 1, H, W), np.float32)
    for c in range(N_CORES):
        # unscramble: row 32k+16h+j, col block cc -> instance j, tile
        # 8*cc + 4*h + k
        scr = np.asarray(res.results[c]["out"], dtype=np.float32).reshape(
            4, 2, PER, NBLK, F
        )  # k, h, j, cc, pix
        full = np.transpose(scr, (2, 3, 1, 0, 4)).reshape(PER, NBLK * 8 * F)
        out[c * PER : (c + 1) * PER, 0] = full[:, :P].reshape(PER, H, W)
    return out, res


def kernel(features, params, num_insts=None, **_ignored):
    out, _ = _run(features, params, trace=False)
    return out
